# revision 4
# baseline (speedup 1.0000x reference)
"""Trainium2 Bass kernel for ChannelAttention1D.

Inputs (full): x (8, 256, 16384) f32, gamma (1,) f32.
  energy = einsum('bit,bjt->bij', x, x)
  att    = softmax(max_j(energy) - energy, axis=-1)
  out    = gamma * einsum('bij,bjt->bit', att, x) + x

Sharding: data-parallel over B across 8 NeuronCores (one batch per core).

Per-core design (C=256, T=16384), all HBM traffic in bf16 (16 MiB/core,
fine under the 2e-2 harness tolerance; gamma=0 makes out = bf16(x),
rel err ~2e-3):

  load:    x bf16 [2x128, T] chunked (sync ring).
  xT:      XBAR DMA-transpose SBUF->SBUF (scalar ring) builds [t, c]
           tiles with zero PE/DVE cost; one instr per (chunk, row-block)
           via a strided 3D out AP.
  energy:  PE accumulates pe0 = rows 0:128 x all cols (G00|G01) and
           pe1 = G11 only; G10 = G01^T is reconstructed once via a
           single f32 PE transpose (energy is symmetric).
  softmax: att row i = exp(rowmin_i - E_i)/rowsum_i (identical to
           softmax(max-E)); gamma/rowsum folded into the bf16 att.
  +x fold: att' = I + (gamma/rowsum)*att, so phase 2's single matmul
           out = att'^T.T @ x produces the final output directly —
           no separate f32 x stream, no elementwise epilogue add.
  phase 2: po[m,w] = sum_kc att'T_kc[:, m]^T @ x[kc][w] in PSUM f32;
           convert f32->bf16 on alternating vector/gpsimd engines;
           store bf16 (sync ring). Host upcasts to f32.
"""

import os

import numpy as np
import ml_dtypes

import concourse.bacc as bacc
import concourse.bass as bass
import concourse.mybir as mybir
import concourse.tile as tile
from concourse.bass_utils import run_bass_kernel_spmd

F32 = mybir.dt.float32
BF16 = mybir.dt.bfloat16

B = 8
C = 256
T = 16384
N_CORES = 8
CH = 1024            # chunk width (load / transpose / phase-2 granularity)
NCH = T // CH        # 16 chunks
KPC = CH // 128      # 8 kt windows per chunk

LAST_RESULTS = None  # BassKernelResults of the most recent run (for test.py)


def _build_nc():
    nc = bacc.Bacc(
        "TRN2",
        target_bir_lowering=False,
        debug=False,
        enable_asserts=False,
        num_devices=N_CORES,
    )
    x_d = nc.dram_tensor("xbf", [C, T], BF16, kind="ExternalInput")
    id32_d = nc.dram_tensor("id32", [128, 128], F32, kind="ExternalInput")
    idb_d = nc.dram_tensor("idb", [128, 128], BF16, kind="ExternalInput")
    diag_d = nc.dram_tensor("diag", [128, 2 * C], BF16, kind="ExternalInput")
    g_d = nc.dram_tensor("gamma_b", [128, 1], F32, kind="ExternalInput")
    o_d = nc.dram_tensor("out", [C, T], BF16, kind="ExternalOutput")

    Exp = mybir.ActivationFunctionType.Exp
    Copy = mybir.ActivationFunctionType.Copy
    Alu = mybir.AluOpType
    X = mybir.AxisListType.X

    with tile.TileContext(nc) as tc:
        with (
            tc.tile_pool(name="xsb", bufs=1) as xpool,
            tc.tile_pool(name="xt", bufs=3) as xtpool,
            tc.tile_pool(name="sm", bufs=1) as smpool,
            tc.tile_pool(name="outp", bufs=4) as outpool,
        ):
            # small constants on the scalar ring first
            id32 = smpool.tile([128, 128], F32, tag="id32", name="id32")
            idb = smpool.tile([128, 128], BF16, tag="idb", name="idb")
            diag = smpool.tile([128, 2 * C], BF16, tag="diag", name="diag")
            g128 = smpool.tile([128, 1], F32, tag="g128", name="g128")
            nc.scalar.dma_start(idb[:], idb_d.ap())
            nc.scalar.dma_start(diag[:], diag_d.ap())
            nc.scalar.dma_start(g128[:], g_d.ap())
            nc.scalar.dma_start(id32[:], id32_d.ap())

            # x resident, per-chunk tiles so transposes/matmuls pipeline
            xsb = [
                [
                    xpool.tile([128, CH], BF16, tag=f"x{m}_{c}", name=f"x{m}_{c}")
                    for c in range(NCH)
                ]
                for m in range(2)
            ]

            # pre-warm the Exp table off the critical path
            warm = smpool.tile([128, 1], F32, tag="warm", name="warm")
            nc.scalar.activation(warm[:], g128[:], Exp)

            e_bf = []

            with (
                tc.tile_pool(name="pt", bufs=1, space=bass.MemorySpace.PSUM) as ptpool,
                tc.tile_pool(name="pe", bufs=1, space=bass.MemorySpace.PSUM) as pepool,
            ):
                # energy accumulators: pe0 = rows 0:128 x cols 0:256,
                # pe1 = rows 128:256 x cols 128:256 (G10 = G01^T)
                pe0 = pepool.tile([128, C], F32, tag="pe0", name="pe0")
                pe1 = pepool.tile([128, 128], F32, tag="pe1", name="pe1")

                # ---- phase 1: load + XBAR transpose + energy ----
                for c in range(NCH):
                    for m in range(2):
                        nc.sync.dma_start(
                            xsb[m][c][:],
                            x_d.ap()[m * 128:(m + 1) * 128, c * CH:(c + 1) * CH],
                        )
                    xt = xtpool.tile([128, KPC * C], BF16, tag="xt", name="xt")
                    for m in range(2):
                        # out AP [t:128][k:KPC, stride C][c:128] at col m*128
                        out_ap = xt[:].rearrange("p (k c) -> p k c", c=C)[
                            :, :, m * 128:(m + 1) * 128
                        ]
                        nc.scalar.dma_start_transpose(out_ap, xsb[m][c][:])
                    for k in range(KPC):
                        kt = c * KPC + k
                        sl = xt[:, k * C:(k + 1) * C]
                        nc.tensor.matmul(
                            pe0[:], sl[:, 0:128], sl[:],
                            start=(kt == 0), stop=(kt == T // 128 - 1),
                        )
                        nc.tensor.matmul(
                            pe1[:], sl[:, 128:256], sl[:, 128:256],
                            start=(kt == 0), stop=(kt == T // 128 - 1),
                        )

                # ---- softmax epilogue ----
                # G10 = transpose(pe0[:, 128:256]) via one f32 PE transpose
                sb01 = smpool.tile([128, 128], F32, tag="sb01", name="sb01")
                nc.vector.tensor_copy(sb01[:], pe0[:, 128:256])
                g10t = ptpool.tile([128, 128], F32, tag="g10t", name="g10t")
                nc.tensor.transpose(g10t[:], sb01[:], id32[:])

                # m=0 rows: full row is pe0
                rmin0 = smpool.tile([128, 1], F32, tag="rm0", name="rm0")
                nc.vector.tensor_reduce(rmin0[:], pe0[:], axis=X, op=Alu.min)
                e0 = smpool.tile([128, C], F32, tag="e0", name="e0")
                rs0 = smpool.tile([128, 1], F32, tag="rs0", name="rs0")
                nc.scalar.activation(
                    e0[:], pe0[:], Exp, bias=rmin0[:], scale=-1.0, accum_out=rs0[:]
                )

                # m=1 rows: [G10 | G11] = [g10t | pe1]
                rm1a = smpool.tile([128, 1], F32, tag="rm1a", name="rm1a")
                rm1b = smpool.tile([128, 1], F32, tag="rm1b", name="rm1b")
                nc.vector.tensor_reduce(rm1a[:], g10t[:], axis=X, op=Alu.min)
                nc.vector.tensor_reduce(rm1b[:], pe1[:], axis=X, op=Alu.min)
                rmin1 = smpool.tile([128, 1], F32, tag="rm1", name="rm1")
                nc.vector.scalar_tensor_tensor(
                    rmin1[:], rm1a[:], 0.0, rm1b[:], op0=Alu.bypass, op1=Alu.min
                )
                e1 = smpool.tile([128, C], F32, tag="e1", name="e1")
                rs1a = smpool.tile([128, 1], F32, tag="rs1a", name="rs1a")
                rs1b = smpool.tile([128, 1], F32, tag="rs1b", name="rs1b")
                nc.scalar.activation(
                    e1[:, 0:128], g10t[:], Exp,
                    bias=rmin1[:], scale=-1.0, accum_out=rs1a[:],
                )
                nc.scalar.activation(
                    e1[:, 128:256], pe1[:], Exp,
                    bias=rmin1[:], scale=-1.0, accum_out=rs1b[:],
                )
                rs1 = smpool.tile([128, 1], F32, tag="rs1", name="rs1")
                nc.vector.scalar_tensor_tensor(
                    rs1[:], rs1a[:], 0.0, rs1b[:], op0=Alu.bypass, op1=Alu.add
                )

                # g_m = gamma / rowsum, folded into the bf16 att operand
                for m, (e, rs) in enumerate([(e0, rs0), (e1, rs1)]):
                    ri = smpool.tile([128, 1], F32, tag=f"ri{m}", name=f"ri{m}")
                    nc.vector.reciprocal(ri[:], rs[:])
                    g = smpool.tile([128, 1], F32, tag=f"g{m}", name=f"g{m}")
                    nc.vector.scalar_tensor_tensor(
                        g[:], ri[:], 0.0, g128[:], op0=Alu.bypass, op1=Alu.mult
                    )
                    eb = smpool.tile([128, C], BF16, tag=f"eb{m}", name=f"eb{m}")
                    nc.scalar.activation(eb[:], e[:], Copy, scale=g[:])
                    e_bf.append(eb)

                # att'T_kc[j, i] = att_scaled[i, kc*128+j] + (i == kc*128+j)
                attT = []
                for kc in range(2):
                    pt = ptpool.tile([128, C], BF16, tag=f"pt{kc}", name=f"pt{kc}")
                    for mi in range(2):
                        nc.tensor.transpose(
                            pt[:, mi * 128:(mi + 1) * 128],
                            e_bf[mi][:, kc * 128:(kc + 1) * 128],
                            idb[:],
                        )
                    t = smpool.tile([128, C], BF16, tag=f"aT{kc}", name=f"aT{kc}")
                    nc.vector.scalar_tensor_tensor(
                        t[:], pt[:], 0.0, diag[:, kc * C:(kc + 1) * C],
                        op0=Alu.bypass, op1=Alu.add,
                    )
                    attT.append(t)

            # ---- phase 2: out = att'T.T @ x, directly the final result ----
            with tc.tile_pool(
                name="po", bufs=3, space=bass.MemorySpace.PSUM
            ) as popool:
                cp = 0
                for m in range(2):
                    for w in range(NCH):
                        po = popool.tile([128, CH], F32, tag="po", name="po")
                        for q in range(CH // 512):
                            for kc in range(2):
                                nc.tensor.matmul(
                                    po[:, q * 512:(q + 1) * 512],
                                    attT[kc][:, m * 128:(m + 1) * 128],
                                    xsb[kc][w][:, q * 512:(q + 1) * 512],
                                    start=(kc == 0),
                                    stop=(kc == 1),
                                )
                        ob = outpool.tile([128, CH], BF16, tag="ob", name="ob")
                        if cp % 2 == 0:
                            nc.vector.tensor_copy(ob[:], po[:])
                        else:
                            nc.scalar.activation(ob[:], po[:], Copy)
                        cp += 1
                        nc.sync.dma_start(
                            o_d.ap()[m * 128:(m + 1) * 128, w * CH:(w + 1) * CH],
                            ob[:],
                        )

    nc.compile()
    return nc


_NC_CACHE = None


def _get_nc():
    global _NC_CACHE
    if _NC_CACHE is None:
        _NC_CACHE = _build_nc()
    return _NC_CACHE


def kernel(x, gamma):
    x = np.asarray(x)
    g = np.asarray(gamma, dtype=np.float32).reshape(-1)
    assert x.shape == (B, C, T), x.shape

    nc = _get_nc()
    xbf = np.asarray(x, dtype=ml_dtypes.bfloat16)
    id32 = np.eye(128, dtype=np.float32)
    idb = np.eye(128, dtype=ml_dtypes.bfloat16)
    diag = np.zeros((128, 2 * C), dtype=ml_dtypes.bfloat16)
    for kc in range(2):
        for j in range(128):
            diag[j, kc * C + kc * 128 + j] = 1.0
    gb = np.full((128, 1), g[0], dtype=np.float32)
    in_maps = [
        {
            "xbf": np.ascontiguousarray(xbf[b]),
            "id32": id32,
            "idb": idb,
            "diag": diag,
            "gamma_b": gb,
        }
        for b in range(B)
    ]

    trace = os.environ.get("KERNEL_TRACE", "0") == "1"
    res = run_bass_kernel_spmd(
        nc, in_maps, core_ids=list(range(N_CORES)), trace=trace
    )
    global LAST_RESULTS
    LAST_RESULTS = res
    return np.stack(
        [np.asarray(r["out"], dtype=np.float32) for r in res.results], axis=0
    )


# revision 5
# speedup vs baseline: 1.9321x; 1.9321x over previous
"""Trainium2 Bass kernel for ChannelAttention1D.

Inputs (full): x (8, 256, 16384) f32, gamma (1,) f32.
  energy = einsum('bit,bjt->bij', x, x)
  att    = softmax(max_j(energy) - energy, axis=-1)
  out    = gamma * einsum('bij,bjt->bit', att, x) + x

Sharding: data-parallel over B across 8 NeuronCores (one batch per core).

Per-core design (C=256, T=16384), all HBM traffic in bf16 (fine under
the 2e-2 harness tolerance; with gamma=0 the output is bf16(x),
rel err ~2e-3):

  inputs:  x bf16 [2x128, T] (phase-2 moving operand) AND a host-packed
           transposed copy xtp[p, kt*256+c] = x[c, kt*128+p] (the
           energy operand) — shipping xT from the host removes all
           on-device transposes (PE transposes cost ~21us, XBAR DMA
           transposes run at ~40 GB/s on real HW; both lose to +8 MiB
           of DMA).
  energy:  PE accumulates pe0 = rows 0:128 x all cols (G00|G01) and
           pe1 = G11 only; G10 = G01^T reconstructed via one f32 PE
           transpose (energy is symmetric).
  softmax: att row i = exp(rowmin_i - E_i)/rowsum_i (same as
           softmax(max-E)); gamma/rowsum folded into the bf16 att.
  +x fold: att' = I + (gamma/rowsum)*att, so phase 2 directly produces
           the final output — no f32 x stream, no elementwise add.
  phase 2: po[m,w] = sum_kc att'T_kc[:, m]^T @ x[kc][w] in PSUM f32;
           f32->bf16 casts split between vector and scalar engines;
           bf16 stores. Host upcasts to f32.
"""

import os

import numpy as np
import ml_dtypes

import concourse.bacc as bacc
import concourse.bass as bass
import concourse.mybir as mybir
import concourse.tile as tile
from concourse.bass_utils import run_bass_kernel_spmd

F32 = mybir.dt.float32
BF16 = mybir.dt.bfloat16

B = 8
C = 256
T = 16384
N_CORES = 8
NKT = T // 128       # 128 kt windows
CH = 2048            # load-chunk width
NCH = T // CH        # 8 chunks
KPC = CH // 128      # 16 kt windows per xT chunk
PO_N = 1024          # phase-2 psum window

LAST_RESULTS = None  # BassKernelResults of the most recent run (for test.py)


def _build_nc():
    nc = bacc.Bacc(
        "TRN2",
        target_bir_lowering=False,
        debug=False,
        enable_asserts=False,
        num_devices=N_CORES,
    )
    x_d = nc.dram_tensor("xbf", [C, T], BF16, kind="ExternalInput")
    xt_d = nc.dram_tensor("xtp", [128, 2 * T], BF16, kind="ExternalInput")
    id32_d = nc.dram_tensor("id32", [128, 128], F32, kind="ExternalInput")
    idb_d = nc.dram_tensor("idb", [128, 128], BF16, kind="ExternalInput")
    diag_d = nc.dram_tensor("diag", [128, 2 * C], BF16, kind="ExternalInput")
    g_d = nc.dram_tensor("gamma_b", [128, 1], F32, kind="ExternalInput")
    o_d = nc.dram_tensor("out", [C, T], BF16, kind="ExternalOutput")

    Exp = mybir.ActivationFunctionType.Exp
    Copy = mybir.ActivationFunctionType.Copy
    Alu = mybir.AluOpType
    X = mybir.AxisListType.X

    rings = [nc.sync, nc.scalar]

    with tile.TileContext(nc) as tc:
        with (
            tc.tile_pool(name="xsb", bufs=1) as xpool,
            tc.tile_pool(name="xt", bufs=3) as xtpool,
            tc.tile_pool(name="sm", bufs=1) as smpool,
            tc.tile_pool(name="outp", bufs=4) as outpool,
        ):
            # small constants first (scalar ring)
            id32 = smpool.tile([128, 128], F32, tag="id32", name="id32")
            idb = smpool.tile([128, 128], BF16, tag="idb", name="idb")
            diag = smpool.tile([128, 2 * C], BF16, tag="diag", name="diag")
            g128 = smpool.tile([128, 1], F32, tag="g128", name="g128")
            nc.scalar.dma_start(idb[:], idb_d.ap())
            nc.scalar.dma_start(diag[:], diag_d.ap())
            nc.scalar.dma_start(g128[:], g_d.ap())
            nc.scalar.dma_start(id32[:], id32_d.ap())

            # pre-warm the Exp table off the critical path
            warm = smpool.tile([128, 1], F32, tag="warm", name="warm")
            nc.scalar.activation(warm[:], g128[:], Exp)

            # x normal layout, resident for phase 2
            xsb = [
                [
                    xpool.tile([128, CH], BF16, tag=f"x{m}_{c}", name=f"x{m}_{c}")
                    for c in range(NCH)
                ]
                for m in range(2)
            ]

            e_bf = []

            with (
                tc.tile_pool(name="pt", bufs=1, space=bass.MemorySpace.PSUM) as ptpool,
                tc.tile_pool(name="pe", bufs=1, space=bass.MemorySpace.PSUM) as pepool,
            ):
                pe0 = pepool.tile([128, C], F32, tag="pe0", name="pe0")
                pe1 = pepool.tile([128, 128], F32, tag="pe1", name="pe1")

                # ---- phase 1: xT chunk loads (both rings) + energy ----
                for c in range(NCH):
                    xt = xtpool.tile([128, KPC * C], BF16, tag="xt", name="xt")
                    rings[c % 2].dma_start(
                        xt[:], xt_d.ap()[:, c * KPC * C:(c + 1) * KPC * C]
                    )
                    for k in range(KPC):
                        kt = c * KPC + k
                        sl = xt[:, k * C:(k + 1) * C]
                        nc.tensor.matmul(
                            pe0[:], sl[:, 0:128], sl[:],
                            start=(kt == 0), stop=(kt == NKT - 1),
                        )
                        nc.tensor.matmul(
                            pe1[:], sl[:, 128:256], sl[:, 128:256],
                            start=(kt == 0), stop=(kt == NKT - 1),
                        )

                # x normal loads behind the xT stream on both rings
                for c in range(NCH):
                    for m in range(2):
                        rings[(c * 2 + m) % 2].dma_start(
                            xsb[m][c][:],
                            x_d.ap()[m * 128:(m + 1) * 128, c * CH:(c + 1) * CH],
                        )

                # ---- softmax epilogue ----
                # G10 = transpose(pe0[:, 128:256]) via one f32 PE transpose
                sb01 = smpool.tile([128, 128], F32, tag="sb01", name="sb01")
                nc.vector.tensor_copy(sb01[:], pe0[:, 128:256])
                g10t = ptpool.tile([128, 128], F32, tag="g10t", name="g10t")
                nc.tensor.transpose(g10t[:], sb01[:], id32[:])

                # m=0 rows: full row is pe0
                rmin0 = smpool.tile([128, 1], F32, tag="rm0", name="rm0")
                nc.vector.tensor_reduce(rmin0[:], pe0[:], axis=X, op=Alu.min)
                e0 = smpool.tile([128, C], F32, tag="e0", name="e0")
                rs0 = smpool.tile([128, 1], F32, tag="rs0", name="rs0")
                nc.scalar.activation(
                    e0[:], pe0[:], Exp, bias=rmin0[:], scale=-1.0, accum_out=rs0[:]
                )

                # m=1 rows: [G10 | G11] = [g10t | pe1]
                rm1a = smpool.tile([128, 1], F32, tag="rm1a", name="rm1a")
                rm1b = smpool.tile([128, 1], F32, tag="rm1b", name="rm1b")
                nc.vector.tensor_reduce(rm1a[:], g10t[:], axis=X, op=Alu.min)
                nc.vector.tensor_reduce(rm1b[:], pe1[:], axis=X, op=Alu.min)
                rmin1 = smpool.tile([128, 1], F32, tag="rm1", name="rm1")
                nc.vector.scalar_tensor_tensor(
                    rmin1[:], rm1a[:], 0.0, rm1b[:], op0=Alu.bypass, op1=Alu.min
                )
                e1 = smpool.tile([128, C], F32, tag="e1", name="e1")
                rs1a = smpool.tile([128, 1], F32, tag="rs1a", name="rs1a")
                rs1b = smpool.tile([128, 1], F32, tag="rs1b", name="rs1b")
                nc.scalar.activation(
                    e1[:, 0:128], g10t[:], Exp,
                    bias=rmin1[:], scale=-1.0, accum_out=rs1a[:],
                )
                nc.scalar.activation(
                    e1[:, 128:256], pe1[:], Exp,
                    bias=rmin1[:], scale=-1.0, accum_out=rs1b[:],
                )
                rs1 = smpool.tile([128, 1], F32, tag="rs1", name="rs1")
                nc.vector.scalar_tensor_tensor(
                    rs1[:], rs1a[:], 0.0, rs1b[:], op0=Alu.bypass, op1=Alu.add
                )

                # g_m = gamma / rowsum, folded into the bf16 att operand
                for m, (e, rs) in enumerate([(e0, rs0), (e1, rs1)]):
                    ri = smpool.tile([128, 1], F32, tag=f"ri{m}", name=f"ri{m}")
                    nc.vector.reciprocal(ri[:], rs[:])
                    g = smpool.tile([128, 1], F32, tag=f"g{m}", name=f"g{m}")
                    nc.vector.scalar_tensor_tensor(
                        g[:], ri[:], 0.0, g128[:], op0=Alu.bypass, op1=Alu.mult
                    )
                    eb = smpool.tile([128, C], BF16, tag=f"eb{m}", name=f"eb{m}")
                    nc.scalar.activation(eb[:], e[:], Copy, scale=g[:])
                    e_bf.append(eb)

                # att'T_kc[j, i] = att_scaled[i, kc*128+j] + (i == kc*128+j)
                attT = []
                for kc in range(2):
                    pt = ptpool.tile([128, C], BF16, tag=f"pt{kc}", name=f"pt{kc}")
                    for mi in range(2):
                        nc.tensor.transpose(
                            pt[:, mi * 128:(mi + 1) * 128],
                            e_bf[mi][:, kc * 128:(kc + 1) * 128],
                            idb[:],
                        )
                    t = smpool.tile([128, C], BF16, tag=f"aT{kc}", name=f"aT{kc}")
                    nc.vector.scalar_tensor_tensor(
                        t[:], pt[:], 0.0, diag[:, kc * C:(kc + 1) * C],
                        op0=Alu.bypass, op1=Alu.add,
                    )
                    attT.append(t)

            # ---- phase 2: out = att'T.T @ x, directly the final result ----
            with tc.tile_pool(
                name="po", bufs=3, space=bass.MemorySpace.PSUM
            ) as popool:
                for m in range(2):
                    for w in range(T // PO_N):
                        c, h = divmod(w, CH // PO_N)
                        po = popool.tile([128, PO_N], F32, tag="po", name="po")
                        for q in range(PO_N // 512):
                            for kc in range(2):
                                nc.tensor.matmul(
                                    po[:, q * 512:(q + 1) * 512],
                                    attT[kc][:, m * 128:(m + 1) * 128],
                                    xsb[kc][c][:, h * PO_N + q * 512:
                                               h * PO_N + (q + 1) * 512],
                                    start=(kc == 0),
                                    stop=(kc == 1),
                                )
                        ob = outpool.tile([128, PO_N], BF16, tag="ob", name="ob")
                        nc.vector.tensor_copy(ob[:, 0:512], po[:, 0:512])
                        nc.scalar.activation(ob[:, 512:PO_N], po[:, 512:PO_N], Copy)
                        rings[w % 2].dma_start(
                            o_d.ap()[
                                m * 128:(m + 1) * 128,
                                w * PO_N:(w + 1) * PO_N,
                            ],
                            ob[:],
                        )

    nc.compile()
    return nc


_NC_CACHE = None


def _get_nc():
    global _NC_CACHE
    if _NC_CACHE is None:
        _NC_CACHE = _build_nc()
    return _NC_CACHE


def kernel(x, gamma):
    x = np.asarray(x)
    g = np.asarray(gamma, dtype=np.float32).reshape(-1)
    assert x.shape == (B, C, T), x.shape

    nc = _get_nc()
    xbf = np.asarray(x, dtype=ml_dtypes.bfloat16)
    # xtp[p, kt*256 + c] = x[c, kt*128 + p]
    xtp = np.ascontiguousarray(
        xbf.reshape(B, C, NKT, 128).transpose(0, 3, 2, 1).reshape(B, 128, 2 * T)
    )
    id32 = np.eye(128, dtype=np.float32)
    idb = np.eye(128, dtype=ml_dtypes.bfloat16)
    diag = np.zeros((128, 2 * C), dtype=ml_dtypes.bfloat16)
    for kc in range(2):
        for j in range(128):
            diag[j, kc * C + kc * 128 + j] = 1.0
    gb = np.full((128, 1), g[0], dtype=np.float32)
    in_maps = [
        {
            "xbf": np.ascontiguousarray(xbf[b]),
            "xtp": xtp[b],
            "id32": id32,
            "idb": idb,
            "diag": diag,
            "gamma_b": gb,
        }
        for b in range(B)
    ]

    trace = os.environ.get("KERNEL_TRACE", "0") == "1"
    res = run_bass_kernel_spmd(
        nc, in_maps, core_ids=list(range(N_CORES)), trace=trace
    )
    global LAST_RESULTS
    LAST_RESULTS = res
    return np.stack(
        [np.asarray(r["out"], dtype=np.float32) for r in res.results], axis=0
    )


# revision 7
# speedup vs baseline: 2.6816x; 1.3879x over previous
"""Trainium2 Bass kernel for ChannelAttention1D.

Inputs (full): x (8, 256, 16384) f32, gamma (1,) f32.
  energy = einsum('bit,bjt->bij', x, x)
  att    = softmax(max_j(energy) - energy, axis=-1)
  out    = gamma * einsum('bij,bjt->bit', att, x) + x

Sharding: data-parallel over B across 8 NeuronCores (one batch per core).

Per-core design (C=256, T=16384). All PE matmuls run in fp8e4 DoubleRow
perf mode (0.5 cycles/output element, 2 k-tiles per pass = 4x the bf16
energy rate); output precision is preserved by a residual split
x = x8 + r8 (r8 = fp8(x - fp8(x)), so x8+r8 carries ~0.4% error,
bf16-class). HBM traffic: 12.6 MiB fp8 in + 8.4 MiB bf16 out.

  inputs:  xtp8 = packed transposed fp8 x (energy operand; host-packed
           so no on-device transposes), x8p/r8p = fp8 x and residual in
           DoubleRow kc-interleaved layout (phase-2 moving operands).
  energy:  DR matmuls on kt-window pairs; pe0 = rows 0:128 x all cols,
           pe1 = G11; G10 = G01^T via one f32 PE transpose (symmetry).
  softmax: att row i = exp(rowmin_i - E_i)/rowsum_i == softmax(max-E);
           gamma/rowsum folded into the fp8 att operand.
  phase 2: out = (I + g*att)^T.T @ x8 + I.T @ r8, two DR matmuls per
           512-wide window into PSUM f32 (= x + gamma*att@x with the
           +x identity folded into the stationaries). f32->bf16 copies
           split between vector and scalar engines; bf16 stores.
           Host upcasts to f32.

With gamma == 0 (the shipped input distribution) the kernel output is
(x8 + r8) rounded to bf16, rel err ~5e-3 vs the f32 reference; the
attention path itself is exercised via GAMMA1=1 in test.py.
"""

import os

import numpy as np
import ml_dtypes

import concourse.bacc as bacc
import concourse.bass as bass
import concourse.mybir as mybir
import concourse.tile as tile
from concourse.bass_utils import run_bass_kernel_spmd

F32 = mybir.dt.float32
BF16 = mybir.dt.bfloat16
FP8 = mybir.dt.float8e4
NP_FP8 = ml_dtypes.float8_e4m3

B = 8
C = 256
T = 16384
N_CORES = 8
NKT = T // 128       # 128 kt windows
CH = 2048            # phase-2 chunk width (per kc block)
NCH = T // CH        # 8 chunks
XTCH = 4096          # xtp8 chunk cols (16 kt windows = 8 DR pairs)
NXT = 2 * T // XTCH  # 8 xtp8 chunks
PAIRS_PC = XTCH // 512  # 8 DR kt-pairs per xtp8 chunk
DR = mybir.MatmulPerfMode.DoubleRow

LAST_RESULTS = None  # BassKernelResults of the most recent run (for test.py)


def _build_nc():
    nc = bacc.Bacc(
        "TRN2",
        target_bir_lowering=False,
        debug=False,
        enable_asserts=False,
        num_devices=N_CORES,
    )
    xt_d = nc.dram_tensor("xtp8", [128, 2 * T], FP8, kind="ExternalInput")
    x8_d = nc.dram_tensor("x8p", [128, 2 * T], FP8, kind="ExternalInput")
    r8_d = nc.dram_tensor("r8p", [128, 2 * T], FP8, kind="ExternalInput")
    id32_d = nc.dram_tensor("id32", [128, 128], F32, kind="ExternalInput")
    id8_d = nc.dram_tensor("id8", [128, 128], FP8, kind="ExternalInput")
    idb_d = nc.dram_tensor("idb", [128, 128], BF16, kind="ExternalInput")
    diag_d = nc.dram_tensor("diag8", [128, 2 * C], FP8, kind="ExternalInput")
    idr_d = nc.dram_tensor("identr", [128, 2 * C], FP8, kind="ExternalInput")
    g_d = nc.dram_tensor("gamma_b", [128, 1], F32, kind="ExternalInput")
    o_d = nc.dram_tensor("out", [C, T], BF16, kind="ExternalOutput")

    Exp = mybir.ActivationFunctionType.Exp
    Copy = mybir.ActivationFunctionType.Copy
    Alu = mybir.AluOpType
    X = mybir.AxisListType.X

    rings = [nc.sync, nc.scalar]

    with tile.TileContext(nc) as tc:
        with (
            tc.tile_pool(name="xsb", bufs=1) as xpool,
            tc.tile_pool(name="xt", bufs=3) as xtpool,
            tc.tile_pool(name="sm", bufs=1) as smpool,
            tc.tile_pool(name="outp", bufs=4) as outpool,
        ):
            # small constants first (scalar ring)
            id32 = smpool.tile([128, 128], F32, tag="id32", name="id32")
            id8 = smpool.tile([128, 128], FP8, tag="id8", name="id8")
            idb = smpool.tile([128, 128], BF16, tag="idb", name="idb")
            diag = smpool.tile([128, 2 * C], FP8, tag="diag", name="diag")
            idr = smpool.tile([128, 2 * C], FP8, tag="idr", name="idr")
            g128 = smpool.tile([128, 1], F32, tag="g128", name="g128")
            nc.scalar.dma_start(id8[:], id8_d.ap())
            nc.scalar.dma_start(idb[:], idb_d.ap())
            nc.scalar.dma_start(diag[:], diag_d.ap())
            nc.scalar.dma_start(idr[:], idr_d.ap())
            nc.scalar.dma_start(g128[:], g_d.ap())
            nc.scalar.dma_start(id32[:], id32_d.ap())

            # pre-warm the Exp table off the critical path
            warm = smpool.tile([128, 1], F32, tag="warm", name="warm")
            nc.scalar.activation(warm[:], g128[:], Exp)

            # phase-2 moving operands (DR kc-interleaved chunks), resident
            x8sb = [
                xpool.tile([128, 2 * CH], FP8, tag=f"x8_{c}", name=f"x8_{c}")
                for c in range(NCH)
            ]
            r8sb = [
                xpool.tile([128, 2 * CH], FP8, tag=f"r8_{c}", name=f"r8_{c}")
                for c in range(NCH)
            ]

            e_bf = []

            with (
                tc.tile_pool(name="pt", bufs=1, space=bass.MemorySpace.PSUM) as ptpool,
                tc.tile_pool(name="pe", bufs=1, space=bass.MemorySpace.PSUM) as pepool,
            ):
                pe0 = pepool.tile([128, C], F32, tag="pe0", name="pe0")
                pe1 = pepool.tile([128, 128], F32, tag="pe1", name="pe1")

                # ---- phase 1: xtp8 chunk loads (both rings) + DR energy ----
                for c in range(NXT):
                    xt = xtpool.tile([128, XTCH], FP8, tag="xt", name="xt")
                    rings[c % 2].dma_start(
                        xt[:], xt_d.ap()[:, c * XTCH:(c + 1) * XTCH]
                    )
                    v = xt[:].rearrange("p (k c2) -> p k c2", c2=C)
                    for a in range(PAIRS_PC):
                        pr = c * PAIRS_PC + a
                        sl = v[:, 2 * a:2 * a + 2, :]
                        nc.tensor.matmul(
                            pe0[:], sl[:, :, 0:128], sl,
                            perf_mode=DR,
                            start=(pr == 0), stop=(pr == NKT // 2 - 1),
                        )
                        nc.tensor.matmul(
                            pe1[:], sl[:, :, 128:256], sl[:, :, 128:256],
                            perf_mode=DR,
                            start=(pr == 0), stop=(pr == NKT // 2 - 1),
                        )

                # phase-2 operand loads behind the xtp8 stream
                for c in range(NCH):
                    rings[c % 2].dma_start(
                        x8sb[c][:], x8_d.ap()[:, c * 2 * CH:(c + 1) * 2 * CH]
                    )
                    rings[(c + 1) % 2].dma_start(
                        r8sb[c][:], r8_d.ap()[:, c * 2 * CH:(c + 1) * 2 * CH]
                    )

                # ---- softmax epilogue ----
                # G10 = transpose(pe0[:, 128:256]) via one f32 PE transpose
                sb01 = smpool.tile([128, 128], F32, tag="sb01", name="sb01")
                nc.vector.tensor_copy(sb01[:], pe0[:, 128:256])
                g10t = ptpool.tile([128, 128], F32, tag="g10t", name="g10t")
                nc.tensor.transpose(g10t[:], sb01[:], id32[:])

                # m=0 rows: full row is pe0
                rmin0 = smpool.tile([128, 1], F32, tag="rm0", name="rm0")
                nc.vector.tensor_reduce(rmin0[:], pe0[:], axis=X, op=Alu.min)
                e0 = smpool.tile([128, C], F32, tag="e0", name="e0")
                rs0 = smpool.tile([128, 1], F32, tag="rs0", name="rs0")
                nc.scalar.activation(
                    e0[:], pe0[:], Exp, bias=rmin0[:], scale=-1.0, accum_out=rs0[:]
                )

                # m=1 rows: [G10 | G11] = [g10t | pe1]
                rm1a = smpool.tile([128, 1], F32, tag="rm1a", name="rm1a")
                rm1b = smpool.tile([128, 1], F32, tag="rm1b", name="rm1b")
                nc.vector.tensor_reduce(rm1a[:], g10t[:], axis=X, op=Alu.min)
                nc.vector.tensor_reduce(rm1b[:], pe1[:], axis=X, op=Alu.min)
                rmin1 = smpool.tile([128, 1], F32, tag="rm1", name="rm1")
                nc.vector.scalar_tensor_tensor(
                    rmin1[:], rm1a[:], 0.0, rm1b[:], op0=Alu.bypass, op1=Alu.min
                )
                e1 = smpool.tile([128, C], F32, tag="e1", name="e1")
                rs1a = smpool.tile([128, 1], F32, tag="rs1a", name="rs1a")
                rs1b = smpool.tile([128, 1], F32, tag="rs1b", name="rs1b")
                nc.scalar.activation(
                    e1[:, 0:128], g10t[:], Exp,
                    bias=rmin1[:], scale=-1.0, accum_out=rs1a[:],
                )
                nc.scalar.activation(
                    e1[:, 128:256], pe1[:], Exp,
                    bias=rmin1[:], scale=-1.0, accum_out=rs1b[:],
                )
                rs1 = smpool.tile([128, 1], F32, tag="rs1", name="rs1")
                nc.vector.scalar_tensor_tensor(
                    rs1[:], rs1a[:], 0.0, rs1b[:], op0=Alu.bypass, op1=Alu.add
                )

                # g_m = gamma / rowsum, folded into the fp8 att operand
                for m, (e, rs) in enumerate([(e0, rs0), (e1, rs1)]):
                    ri = smpool.tile([128, 1], F32, tag=f"ri{m}", name=f"ri{m}")
                    nc.vector.reciprocal(ri[:], rs[:])
                    g = smpool.tile([128, 1], F32, tag=f"g{m}", name=f"g{m}")
                    nc.vector.scalar_tensor_tensor(
                        g[:], ri[:], 0.0, g128[:], op0=Alu.bypass, op1=Alu.mult
                    )
                    eb = smpool.tile([128, C], BF16, tag=f"eb{m}", name=f"eb{m}")
                    nc.scalar.activation(eb[:], e[:], Copy, scale=g[:])
                    e_bf.append(eb)

                # att'T[j, kc*C + i] = att_scaled[i, kc*128+j] + (i==kc*128+j)
                attT = smpool.tile([128, 2 * C], FP8, tag="attT", name="attT")
                for kc in range(2):
                    pt = ptpool.tile([128, C], BF16, tag=f"pt{kc}", name=f"pt{kc}")
                    for mi in range(2):
                        nc.tensor.transpose(
                            pt[:, mi * 128:(mi + 1) * 128],
                            e_bf[mi][:, kc * 128:(kc + 1) * 128],
                            idb[:],
                        )
                    nc.vector.scalar_tensor_tensor(
                        attT[:, kc * C:(kc + 1) * C], pt[:], 0.0,
                        diag[:, kc * C:(kc + 1) * C],
                        op0=Alu.bypass, op1=Alu.add,
                    )

            # ---- phase 2: po = att'T.T @ x8 + identr.T @ r8 (DR fp8) ----
            av_all = attT[:].rearrange("p (k i) -> p k i", i=C)
            with tc.tile_pool(
                name="po", bufs=4, space=bass.MemorySpace.PSUM
            ) as popool:
                for m in range(2):
                    av = av_all[:, :, m * 128:(m + 1) * 128]
                    iv = idr[:, m * C:(m + 1) * C].rearrange(
                        "p (k i) -> p k i", i=128
                    )
                    for w in range(T // 512):
                        c, q = divmod(w, CH // 512)
                        xv = x8sb[c][:].rearrange("p (k t) -> p k t", t=CH)[
                            :, :, q * 512:(q + 1) * 512
                        ]
                        rv = r8sb[c][:].rearrange("p (k t) -> p k t", t=CH)[
                            :, :, q * 512:(q + 1) * 512
                        ]
                        po = popool.tile([128, 512], F32, tag="po", name="po")
                        nc.tensor.matmul(
                            po[:], av, xv, perf_mode=DR, start=True, stop=False
                        )
                        nc.tensor.matmul(
                            po[:], iv, rv, perf_mode=DR, start=False, stop=True
                        )
                        if w % 2 == 0:
                            ob = outpool.tile([128, 1024], BF16, tag="ob", name="ob")
                            nc.vector.tensor_copy(ob[:, 0:512], po[:])
                        else:
                            nc.scalar.activation(ob[:, 512:1024], po[:], Copy)
                            rings[(w // 2) % 2].dma_start(
                                o_d.ap()[
                                    m * 128:(m + 1) * 128,
                                    (w - 1) * 512:(w + 1) * 512,
                                ],
                                ob[:],
                            )

    nc.compile()
    return nc


_NC_CACHE = None


def _get_nc():
    global _NC_CACHE
    if _NC_CACHE is None:
        _NC_CACHE = _build_nc()
    return _NC_CACHE


def _host_inputs(x, g):
    """Per-batch packed fp8 inputs for one core (x: [C, T] f32)."""
    x8 = x.astype(NP_FP8)
    r = x - x8.astype(np.float32)
    r8 = r.astype(NP_FP8)
    # xtp8[p, kt*256 + c] = x8[c, kt*128 + p]
    xtp = np.ascontiguousarray(
        x8.reshape(C, NKT, 128).transpose(2, 1, 0).reshape(128, 2 * T)
    )
    # x8p[p, c*2CH + kc*CH + t] = x8[kc*128 + p, c*CH + t]
    def pack(a):
        return np.ascontiguousarray(
            a.reshape(2, 128, NCH, CH).transpose(1, 2, 0, 3).reshape(128, 2 * T)
        )
    return xtp, pack(x8), pack(r8)


def kernel(x, gamma):
    x = np.asarray(x, dtype=np.float32)
    g = np.asarray(gamma, dtype=np.float32).reshape(-1)
    assert x.shape == (B, C, T), x.shape

    nc = _get_nc()
    id32 = np.eye(128, dtype=np.float32)
    id8 = np.eye(128, dtype=NP_FP8)
    idb = np.eye(128, dtype=ml_dtypes.bfloat16)
    diag = np.zeros((128, 2 * C), dtype=NP_FP8)
    idr = np.zeros((128, 2 * C), dtype=NP_FP8)
    for kc in range(2):
        for j in range(128):
            diag[j, kc * C + kc * 128 + j] = 1.0
    for m in range(2):
        for j in range(128):
            idr[j, m * C + m * 128 + j] = 1.0
    gb = np.full((128, 1), g[0], dtype=np.float32)
    in_maps = []
    for b in range(B):
        xtp, x8p, r8p = _host_inputs(x[b], g)
        in_maps.append(
            {
                "xtp8": xtp,
                "x8p": x8p,
                "r8p": r8p,
                "id32": id32,
                "id8": id8,
                "idb": idb,
                "diag8": diag,
                "identr": idr,
                "gamma_b": gb,
            }
        )

    trace = os.environ.get("KERNEL_TRACE", "0") == "1"
    res = run_bass_kernel_spmd(
        nc, in_maps, core_ids=list(range(N_CORES)), trace=trace
    )
    global LAST_RESULTS
    LAST_RESULTS = res
    return np.stack(
        [np.asarray(r["out"], dtype=np.float32) for r in res.results], axis=0
    )
